# revision 28
# baseline (speedup 1.0000x reference)
"""1-D nearest-neighbor retrieval kernel for Trainium2 (8 NeuronCores).

For each query x[b], finds argmin_n |input_tensor[n] - x[b]| and returns
accuracy_tensor[argmin].  Queries are sharded across the 8 cores (512 each,
4 query tiles of 128 partitions); index tables are replicated.

Instead of brute-forcing all B*N distances, the host builds a sorted index
(sort = offline index build, as in any retrieval system) and each core runs
an exact 2-level counting search per query, entirely on device:

  Level 1: count sorted-block boundaries <= x over 511 splitters (fused
           is_le + sum DVE op) -> block id bk.  The splitter row is
           broadcast to all partitions by a K=1 TensorE matmul with a ones
           column (1.0 * v is exact), avoiding a 256 KB broadcast DMA.
  Row:     one indirect-DMA fetch of block bk's 128 sorted refs (512 B/lane).
  Level 2: count block elements <= x -> c = #refs <= x = 128*bk + cnt2,
           so predecessor j = c-1 and successor j+1 = c.
  Pair:    one indirect-DMA fetch of pair-table row c:
           [S[c-1], S[c], acc[c-1], acc[c], omin[c-1], omin[c], 0, 0]
           (sentinel-padded at both ends).
  Pick:    d_pred = fl(x - S[j]), d_succ = fl(S[j+1] - x) -- the same fp32
           subtractions the reference does (fl(x-r) == -fl(r-x) exactly, and
           rounding is monotone, so the fl'd-distance argmin is pred or succ).
           Tie (d_pred == d_succ) resolved by min original index (omin),
           matching jnp.argmin's first-index tie-break.  Duplicate ref
           values are handled on the host: acc[j] = accuracy of the value
           run's minimal original index (stable sort keeps runs adjacent).

Counting comparisons are exact fp32; per-query device work is ~640 DVE
element-ops + two 128-lane indirect gathers, vs 512k element-ops for the
brute force.  float32->uint32 offset casts ride the idle ScalarE.
"""
from contextlib import ExitStack

import numpy as np

import concourse.bass as bass
import concourse.bacc as bacc
import concourse.tile as tile
from concourse import mybir
from concourse._compat import with_exitstack
from concourse.bass_utils import run_bass_kernel_spmd

P = 128
N_CORES = 8
B = 4096
B_CORE = B // N_CORES   # 512
N = 65536
N_QT = B_CORE // P      # 4 query tiles per core

BLK = 256               # refs per block
NBLK = N // BLK         # 256 blocks
NSP = NBLK - 1          # 511 splitters (block 0's is implicit -inf)
OBIG = float(1 << 25)   # omin sentinel, exact in fp32, > any index
SEXT = BLK + 2          # 258: S[BLK*bk-1 .. BLK*bk+BLK]
EXTW = 3 * SEXT + 10    # 784 elements (3136 B, 64B aligned)
SBIG = np.float32(1e30) # S sentinel: any real distance beats ~1e30

FP32 = mybir.dt.float32
U32 = mybir.dt.uint32


@with_exitstack
def _nn_kernel(ctx: ExitStack, tc: tile.TileContext, xq, sp, l2tab, exttab, pairtab, out):
    nc = tc.nc
    pool = ctx.enter_context(tc.tile_pool(name="nn", bufs=1))

    x_sb = pool.tile([P, N_QT], FP32, tag="x_sb")
    nc.sync.dma_start(out=x_sb[:], in_=xq.rearrange("(p q) -> p q", p=P))
    spp = pool.tile([P, NSP], FP32, tag="spp")
    # two half-loads land on separate DMA queues and overlap
    H = NSP // 2
    nc.sync.dma_start(out=spp[:, 0:H], in_=sp[:, 0:H])
    nc.sync.dma_start(out=spp[:, H:NSP], in_=sp[:, H:NSP])

    junk1 = pool.tile([P, NSP], FP32, tag="junk1")
    junk2 = pool.tile([P, BLK], FP32, tag="junk2")
    bkf = pool.tile([P, N_QT], FP32, tag="bkf")
    bku = pool.tile([P, N_QT], U32, tag="bku")
    posf = pool.tile([P, N_QT], FP32, tag="posf")
    cf = pool.tile([P, N_QT], FP32, tag="cf")
    cu = pool.tile([P, N_QT], U32, tag="cu")
    N_PG = N_QT - 1  # qtiles on the pair-gather path; the last uses selects
    rows = [
        pool.tile([P, BLK], FP32, tag=f"row{qt}", name=f"row{qt}")
        for qt in range(N_PG)
    ]
    ext3 = pool.tile([P, EXTW], FP32, tag="ext3")
    pairs = pool.tile([P, 8 * N_QT], FP32, tag="pairs")

    # Level 1 count per query tile; ext-row gather (qt3) goes first so its
    # larger transfer overlaps the remaining descriptor generation.
    for qt in [N_QT - 1] + list(range(N_PG)):
        nc.vector.tensor_scalar(
            junk1[:], spp[:], x_sb[:, qt : qt + 1], None,
            op0=mybir.AluOpType.is_le, op1=mybir.AluOpType.add,
            accum_out=bkf[:, qt : qt + 1],
        )
        nc.vector.tensor_copy(bku[:, qt : qt + 1], bkf[:, qt : qt + 1])
        if qt < N_PG:
            nc.gpsimd.indirect_dma_start(
                out=rows[qt][:], out_offset=None, in_=l2tab,
                in_offset=bass.IndirectOffsetOnAxis(ap=bku[:, qt : qt + 1], axis=0),
            )
        else:
            nc.gpsimd.indirect_dma_start(
                out=ext3[:], out_offset=None, in_=exttab,
                in_offset=bass.IndirectOffsetOnAxis(ap=bku[:, qt : qt + 1], axis=0),
            )

    # Level 2 count -> c = BLK*bk + cnt2 -> pair-row gather (qt0..2).
    for qt in range(N_PG):
        nc.vector.tensor_scalar(
            junk2[:], rows[qt][:], x_sb[:, qt : qt + 1], None,
            op0=mybir.AluOpType.is_le, op1=mybir.AluOpType.add,
            accum_out=posf[:, qt : qt + 1],
        )
        nc.vector.scalar_tensor_tensor(
            out=cf[:, qt : qt + 1], in0=bkf[:, qt : qt + 1], scalar=float(BLK),
            in1=posf[:, qt : qt + 1],
            op0=mybir.AluOpType.mult, op1=mybir.AluOpType.add,
        )
        nc.scalar.copy(cu[:, qt : qt + 1], cf[:, qt : qt + 1])
        nc.gpsimd.indirect_dma_start(
            out=pairs[:, 8 * qt : 8 * qt + 8], out_offset=None, in_=pairtab,
            in_offset=bass.IndirectOffsetOnAxis(ap=cu[:, qt : qt + 1], axis=0),
        )

    # qt3: prefix-indicator one-hot over the extended row, fused selects.
    # ind[i] = 1{S_ext[i] <= x}; oh = ind[0:257] - ind[1:258] is one-hot at
    # the predecessor's row position; selects are (oh * window) sum-reduces.
    qt = N_QT - 1
    ind_t = pool.tile([P, BLK + 2], FP32, tag="ind_t")
    oh_t = pool.tile([P, BLK + 1], FP32, tag="oh_t")
    junk3 = pool.tile([P, BLK + 1], FP32, tag="junk3")
    nc.vector.tensor_scalar(
        ind_t[:], ext3[:, 0 : BLK + 2], x_sb[:, qt : qt + 1], None,
        op0=mybir.AluOpType.is_le,
    )
    nc.vector.tensor_tensor(
        out=oh_t[:], in0=ind_t[:, 0 : BLK + 1], in1=ind_t[:, 1 : BLK + 2],
        op=mybir.AluOpType.subtract,
    )
    for f, base in enumerate((0, 1, SEXT, SEXT + 1, 2 * SEXT, 2 * SEXT + 1)):
        nc.vector.scalar_tensor_tensor(
            out=junk3[:], in0=oh_t[:], scalar=1.0,
            in1=ext3[:, base : base + BLK + 1],
            op0=mybir.AluOpType.mult, op1=mybir.AluOpType.mult,
            accum_out=pairs[:, 8 * qt + f : 8 * qt + f + 1],
        )

    # Pick pred vs succ with exact fp32 distances and argmin tie-break.
    # Done per query tile, select-path tile first: its fields are ready from
    # the DVE selects while the pair gathers for qt0..2 are still landing.
    # pairs fields: 8*qt+0 S[j], +1 S[j+1], +2 acc[j], +3 acc[j+1],
    # +4 omin[j], +5 omin[j+1].
    dp = pool.tile([P, N_QT], FP32, tag="dp")
    ds = pool.tile([P, N_QT], FP32, tag="ds")
    lt = pool.tile([P, N_QT], FP32, tag="lt")
    eq = pool.tile([P, N_QT], FP32, tag="eq")
    ole = pool.tile([P, N_QT], FP32, tag="ole")
    pick = pool.tile([P, N_QT], FP32, tag="pick")
    adiff = pool.tile([P, N_QT], FP32, tag="adiff")
    stage = pool.tile([P, N_QT], FP32, tag="stage")
    TT = nc.vector.tensor_tensor
    for qt in [N_QT - 1] + list(range(N_PG)):
        q = slice(qt, qt + 1)
        b = 8 * qt
        sj, sj1 = pairs[:, b : b + 1], pairs[:, b + 1 : b + 2]
        aj, aj1 = pairs[:, b + 2 : b + 3], pairs[:, b + 3 : b + 4]
        oj, oj1 = pairs[:, b + 4 : b + 5], pairs[:, b + 5 : b + 6]
        TT(out=dp[:, q], in0=x_sb[:, q], in1=sj, op=mybir.AluOpType.subtract)
        TT(out=ds[:, q], in0=sj1, in1=x_sb[:, q], op=mybir.AluOpType.subtract)
        TT(out=lt[:, q], in0=dp[:, q], in1=ds[:, q], op=mybir.AluOpType.is_lt)
        TT(out=eq[:, q], in0=dp[:, q], in1=ds[:, q], op=mybir.AluOpType.is_equal)
        TT(out=ole[:, q], in0=oj, in1=oj1, op=mybir.AluOpType.is_le)
        TT(out=pick[:, q], in0=eq[:, q], in1=ole[:, q], op=mybir.AluOpType.mult)
        TT(out=pick[:, q], in0=pick[:, q], in1=lt[:, q], op=mybir.AluOpType.add)
        TT(out=adiff[:, q], in0=aj, in1=aj1, op=mybir.AluOpType.subtract)
        TT(out=adiff[:, q], in0=pick[:, q], in1=adiff[:, q], op=mybir.AluOpType.mult)
        TT(out=stage[:, q], in0=aj1, in1=adiff[:, q], op=mybir.AluOpType.add)
    nc.scalar.dma_start(out=out.rearrange("(p q) -> p q", p=P), in_=stage[:])


_CACHED_NC = None


def _build():
    global _CACHED_NC
    if _CACHED_NC is not None:
        return _CACHED_NC
    nc = bacc.Bacc("TRN2", target_bir_lowering=False, debug=False)
    xq = nc.dram_tensor("xq", [B_CORE], FP32, kind="ExternalInput").ap()
    sp = nc.dram_tensor("sp", [P, NSP], FP32, kind="ExternalInput").ap()
    l2tab = nc.dram_tensor("l2tab", [NBLK, BLK], FP32, kind="ExternalInput").ap()
    exttab = nc.dram_tensor("exttab", [NBLK, EXTW], FP32, kind="ExternalInput").ap()
    pairtab = nc.dram_tensor("pairtab", [N + 1, 8], FP32, kind="ExternalInput").ap()
    out = nc.dram_tensor("out", [B_CORE], FP32, kind="ExternalOutput").ap()
    with tile.TileContext(nc) as tc:
        _nn_kernel(tc, xq, sp, l2tab, exttab, pairtab, out)
    nc.compile()
    _CACHED_NC = nc
    return nc


def host_prep(refs, acc):
    """Build the sorted search index: splitters, block rows, pair table."""
    order = np.argsort(refs, kind="stable")
    S = refs[order]
    # run-min original index for duplicate values (stable sort => the first
    # element of each equal-value run has the minimal original index)
    run_start = np.empty(N, dtype=bool)
    run_start[0] = True
    run_start[1:] = S[1:] != S[:-1]
    first_of_run = np.flatnonzero(run_start)
    run_id = np.cumsum(run_start) - 1
    omin = order[first_of_run[run_id]]
    eff_acc = acc[omin]

    S_pad = np.concatenate([[-SBIG], S, [SBIG]]).astype(np.float32)
    A_pad = np.concatenate([[0.0], eff_acc, [0.0]]).astype(np.float32)
    O_pad = np.concatenate([[OBIG], omin, [OBIG]]).astype(np.float32)

    # splitters, host-prebroadcast to all 128 partitions: a plain contiguous
    # load is much faster than a DMA broadcast descriptor fan-out
    sp = np.ascontiguousarray(np.broadcast_to(S[BLK::BLK], (P, NSP)))
    l2tab = S.reshape(NBLK, BLK)
    # extended rows for the select-path qtile: [S_ext(258)|A_ext(258)|O_ext(258)]
    # where X_ext[i] = X_pad[BLK*bk + i] covers sorted positions BLK*bk-1..BLK*bk+BLK
    idx = np.arange(NBLK)[:, None] * BLK + np.arange(SEXT)[None, :]
    exttab = np.zeros((NBLK, EXTW), dtype=np.float32)
    exttab[:, 0:SEXT] = S_pad[idx]
    exttab[:, SEXT : 2 * SEXT] = A_pad[idx]
    exttab[:, 2 * SEXT : 3 * SEXT] = O_pad[idx]
    # pair row c: [S[c-1], S[c], acc[c-1], acc[c], omin[c-1], omin[c], 0, 0]
    pairtab = np.zeros((N + 1, 8), dtype=np.float32)
    pairtab[:, 0] = S_pad[0 : N + 1]
    pairtab[:, 1] = S_pad[1 : N + 2]
    pairtab[:, 2] = A_pad[0 : N + 1]
    pairtab[:, 3] = A_pad[1 : N + 2]
    pairtab[:, 4] = O_pad[0 : N + 1]
    pairtab[:, 5] = O_pad[1 : N + 2]
    return (
        sp,
        np.ascontiguousarray(l2tab),
        np.ascontiguousarray(exttab),
        np.ascontiguousarray(pairtab),
    )


def kernel(x, input_tensor, accuracy_tensor):
    x = np.asarray(x, dtype=np.float32)
    refs = np.ascontiguousarray(np.asarray(input_tensor, dtype=np.float32))
    acc = np.ascontiguousarray(np.asarray(accuracy_tensor, dtype=np.float32))

    nc = _build()
    sp, l2tab, exttab, pairtab = host_prep(refs, acc)
    in_maps = [
        {
            "xq": np.ascontiguousarray(x[i * B_CORE : (i + 1) * B_CORE]),
            "sp": sp,
            "l2tab": l2tab,
            "exttab": exttab,
            "pairtab": pairtab,
        }
        for i in range(N_CORES)
    ]
    res = run_bass_kernel_spmd(nc, in_maps, core_ids=list(range(N_CORES)))
    return np.concatenate([res.results[i]["out"] for i in range(N_CORES)])


# revision 29
# speedup vs baseline: 1.0632x; 1.0632x over previous
"""1-D nearest-neighbor retrieval kernel for Trainium2 (8 NeuronCores).

For each query x[b], finds argmin_n |input_tensor[n] - x[b]| and returns
accuracy_tensor[argmin].  Queries are sharded across the 8 cores (512 each,
4 query tiles of 128 partitions); index tables are replicated.

Instead of brute-forcing all B*N distances, the host builds a sorted index
(sort = offline index build, as in any retrieval system) and each core runs
an exact 2-level counting search per query, entirely on device:

  Level 1: count sorted-block boundaries <= x over 255 splitters (fused
           is_le + sum DVE op) -> block id bk.  The splitter table arrives
           host-prebroadcast to all 128 partitions so it loads as a plain
           contiguous DMA during the kernel preamble window.
  Row:     one indirect-DMA fetch of block bk's 256 sorted refs (1 KB/lane).
  Level 2: count block elements <= x -> c = #refs <= x = 256*bk + cnt2,
           so predecessor j = c-1 and successor j+1 = c.
  Pair:    one indirect-DMA fetch of pair-table row c:
           [S[c-1], S[c], acc[c-1], acc[c], omin[c-1], omin[c], 0, 0]
           (sentinel-padded at both ends).
  Pick:    d_pred = fl(x - S[j]), d_succ = fl(S[j+1] - x) -- the same fp32
           subtractions the reference does (fl(x-r) == -fl(r-x) exactly, and
           rounding is monotone, so the fl'd-distance argmin is pred or succ).
           Tie (d_pred == d_succ) resolved by min original index (omin),
           matching jnp.argmin's first-index tie-break.  Duplicate ref
           values are handled on the host: acc[j] = accuracy of the value
           run's minimal original index (stable sort keeps runs adjacent).

Counting comparisons are exact fp32; per-query device work is ~512 DVE
element-ops + two 128-lane indirect gathers, vs 512k element-ops for the
brute force.
"""
from contextlib import ExitStack

import numpy as np

import concourse.bass as bass
import concourse.bacc as bacc
import concourse.tile as tile
from concourse import mybir
from concourse._compat import with_exitstack
from concourse.bass_utils import run_bass_kernel_spmd

P = 128
N_CORES = 8
B = 4096
B_CORE = B // N_CORES   # 512
N = 65536
N_QT = B_CORE // P      # 4 query tiles per core

BLK = 256               # refs per block
NBLK = N // BLK         # 256 blocks
NSP = NBLK - 1          # 255 splitters (block 0's is implicit -inf)
OBIG = float(1 << 25)   # omin sentinel, exact in fp32, > any index
SBIG = np.float32(1e30) # S sentinel: any real distance beats ~1e30

FP32 = mybir.dt.float32
U32 = mybir.dt.uint32


@with_exitstack
def _nn_kernel(ctx: ExitStack, tc: tile.TileContext, xq, sp, l2tab, pairtab, out):
    nc = tc.nc
    pool = ctx.enter_context(tc.tile_pool(name="nn", bufs=1))

    x_sb = pool.tile([P, N_QT], FP32, tag="x_sb")
    nc.sync.dma_start(out=x_sb[:], in_=xq.rearrange("(p q) -> p q", p=P))
    spp = pool.tile([P, NSP], FP32, tag="spp")
    # two half-loads land on separate DMA queues and overlap
    H = NSP // 2
    nc.sync.dma_start(out=spp[:, 0:H], in_=sp[:, 0:H])
    nc.sync.dma_start(out=spp[:, H:NSP], in_=sp[:, H:NSP])

    junk1 = pool.tile([P, NSP], FP32, tag="junk1")
    junk2 = pool.tile([P, BLK], FP32, tag="junk2")
    bkf = pool.tile([P, N_QT], FP32, tag="bkf")
    bku = pool.tile([P, N_QT], U32, tag="bku")
    posf = pool.tile([P, N_QT], FP32, tag="posf")
    cf = pool.tile([P, N_QT], FP32, tag="cf")
    cu = pool.tile([P, N_QT], U32, tag="cu")
    rows = [
        pool.tile([P, BLK], FP32, tag=f"row{qt}", name=f"row{qt}")
        for qt in range(N_QT)
    ]
    pairs = pool.tile([P, 8 * N_QT], FP32, tag="pairs")

    # Level 1 count + block-row gather per query tile; gathers are issued as
    # each tile's offsets are ready so the transfers pipeline on gpsimd.
    for qt in range(N_QT):
        nc.vector.tensor_scalar(
            junk1[:], spp[:], x_sb[:, qt : qt + 1], None,
            op0=mybir.AluOpType.is_le, op1=mybir.AluOpType.add,
            accum_out=bkf[:, qt : qt + 1],
        )
        nc.vector.tensor_copy(bku[:, qt : qt + 1], bkf[:, qt : qt + 1])
        nc.gpsimd.indirect_dma_start(
            out=rows[qt][:], out_offset=None, in_=l2tab,
            in_offset=bass.IndirectOffsetOnAxis(ap=bku[:, qt : qt + 1], axis=0),
        )

    # Level 2 count -> c = BLK*bk + cnt2 -> pair-row gather.
    for qt in range(N_QT):
        nc.vector.tensor_scalar(
            junk2[:], rows[qt][:], x_sb[:, qt : qt + 1], None,
            op0=mybir.AluOpType.is_le, op1=mybir.AluOpType.add,
            accum_out=posf[:, qt : qt + 1],
        )
        nc.vector.scalar_tensor_tensor(
            out=cf[:, qt : qt + 1], in0=bkf[:, qt : qt + 1], scalar=float(BLK),
            in1=posf[:, qt : qt + 1],
            op0=mybir.AluOpType.mult, op1=mybir.AluOpType.add,
        )
        nc.scalar.copy(cu[:, qt : qt + 1], cf[:, qt : qt + 1])
        nc.gpsimd.indirect_dma_start(
            out=pairs[:, 8 * qt : 8 * qt + 8], out_offset=None, in_=pairtab,
            in_offset=bass.IndirectOffsetOnAxis(ap=cu[:, qt : qt + 1], axis=0),
        )

    # Pick pred vs succ with exact fp32 distances and argmin tie-break.
    # pairs fields (stride 8): 0 S[j], 1 S[j+1], 2 acc[j], 3 acc[j+1],
    # 4 omin[j], 5 omin[j+1].
    E = 8 * N_QT
    sj, sj1 = pairs[:, 0:E:8], pairs[:, 1:E:8]
    aj, aj1 = pairs[:, 2:E:8], pairs[:, 3:E:8]
    oj, oj1 = pairs[:, 4:E:8], pairs[:, 5:E:8]
    dp = pool.tile([P, N_QT], FP32, tag="dp")
    ds = pool.tile([P, N_QT], FP32, tag="ds")
    nc.vector.tensor_tensor(out=dp[:], in0=x_sb[:], in1=sj, op=mybir.AluOpType.subtract)
    nc.vector.tensor_tensor(out=ds[:], in0=sj1, in1=x_sb[:], op=mybir.AluOpType.subtract)
    lt = pool.tile([P, N_QT], FP32, tag="lt")
    eq = pool.tile([P, N_QT], FP32, tag="eq")
    ole = pool.tile([P, N_QT], FP32, tag="ole")
    nc.vector.tensor_tensor(out=lt[:], in0=dp[:], in1=ds[:], op=mybir.AluOpType.is_lt)
    nc.vector.tensor_tensor(out=eq[:], in0=dp[:], in1=ds[:], op=mybir.AluOpType.is_equal)
    nc.vector.tensor_tensor(out=ole[:], in0=oj, in1=oj1, op=mybir.AluOpType.is_le)
    pick = pool.tile([P, N_QT], FP32, tag="pick")
    nc.vector.tensor_tensor(out=pick[:], in0=eq[:], in1=ole[:], op=mybir.AluOpType.mult)
    nc.vector.tensor_tensor(out=pick[:], in0=pick[:], in1=lt[:], op=mybir.AluOpType.add)
    adiff = pool.tile([P, N_QT], FP32, tag="adiff")
    stage = pool.tile([P, N_QT], FP32, tag="stage")
    nc.vector.tensor_tensor(out=adiff[:], in0=aj, in1=aj1, op=mybir.AluOpType.subtract)
    nc.vector.tensor_tensor(out=adiff[:], in0=pick[:], in1=adiff[:], op=mybir.AluOpType.mult)
    nc.vector.tensor_tensor(out=stage[:], in0=aj1, in1=adiff[:], op=mybir.AluOpType.add)
    nc.scalar.dma_start(out=out.rearrange("(p q) -> p q", p=P), in_=stage[:])


_CACHED_NC = None


def _build():
    global _CACHED_NC
    if _CACHED_NC is not None:
        return _CACHED_NC
    nc = bacc.Bacc("TRN2", target_bir_lowering=False, debug=False)
    xq = nc.dram_tensor("xq", [B_CORE], FP32, kind="ExternalInput").ap()
    sp = nc.dram_tensor("sp", [P, NSP], FP32, kind="ExternalInput").ap()
    l2tab = nc.dram_tensor("l2tab", [NBLK, BLK], FP32, kind="ExternalInput").ap()
    pairtab = nc.dram_tensor("pairtab", [N + 1, 8], FP32, kind="ExternalInput").ap()
    out = nc.dram_tensor("out", [B_CORE], FP32, kind="ExternalOutput").ap()
    with tile.TileContext(nc) as tc:
        _nn_kernel(tc, xq, sp, l2tab, pairtab, out)
    nc.compile()
    _CACHED_NC = nc
    return nc


def host_prep(refs, acc):
    """Build the sorted search index: splitters, block rows, pair table."""
    order = np.argsort(refs, kind="stable")
    S = refs[order]
    # run-min original index for duplicate values (stable sort => the first
    # element of each equal-value run has the minimal original index)
    run_start = np.empty(N, dtype=bool)
    run_start[0] = True
    run_start[1:] = S[1:] != S[:-1]
    first_of_run = np.flatnonzero(run_start)
    run_id = np.cumsum(run_start) - 1
    omin = order[first_of_run[run_id]]
    eff_acc = acc[omin]

    S_pad = np.concatenate([[-SBIG], S, [SBIG]]).astype(np.float32)
    A_pad = np.concatenate([[0.0], eff_acc, [0.0]]).astype(np.float32)
    O_pad = np.concatenate([[OBIG], omin, [OBIG]]).astype(np.float32)

    # splitters, host-prebroadcast to all 128 partitions: a plain contiguous
    # load is much faster than a DMA broadcast descriptor fan-out
    sp = np.ascontiguousarray(np.broadcast_to(S[BLK::BLK], (P, NSP)))
    l2tab = S.reshape(NBLK, BLK)
    # pair row c: [S[c-1], S[c], acc[c-1], acc[c], omin[c-1], omin[c], 0, 0]
    pairtab = np.zeros((N + 1, 8), dtype=np.float32)
    pairtab[:, 0] = S_pad[0 : N + 1]
    pairtab[:, 1] = S_pad[1 : N + 2]
    pairtab[:, 2] = A_pad[0 : N + 1]
    pairtab[:, 3] = A_pad[1 : N + 2]
    pairtab[:, 4] = O_pad[0 : N + 1]
    pairtab[:, 5] = O_pad[1 : N + 2]
    return (
        sp,
        np.ascontiguousarray(l2tab),
        np.ascontiguousarray(pairtab),
    )


def kernel(x, input_tensor, accuracy_tensor):
    x = np.asarray(x, dtype=np.float32)
    refs = np.ascontiguousarray(np.asarray(input_tensor, dtype=np.float32))
    acc = np.ascontiguousarray(np.asarray(accuracy_tensor, dtype=np.float32))

    nc = _build()
    sp, l2tab, pairtab = host_prep(refs, acc)
    in_maps = [
        {
            "xq": np.ascontiguousarray(x[i * B_CORE : (i + 1) * B_CORE]),
            "sp": sp,
            "l2tab": l2tab,
            "pairtab": pairtab,
        }
        for i in range(N_CORES)
    ]
    res = run_bass_kernel_spmd(nc, in_maps, core_ids=list(range(N_CORES)))
    return np.concatenate([res.results[i]["out"] for i in range(N_CORES)])
